# revision 1
# baseline (speedup 1.0000x reference)
"""GCN block (3x GCNConv(128,128) + relu + global_mean_pool) on 8 trn2 cores.

Strategy (graph partition by destination node):
  - Nodes are split into 8 contiguous shards (12544 = 98*128 per core).
  - Each core aggregates messages for its own destination nodes only:
    edges (+self loops) are bucketed by (dst tile, src quarter), and each
    core gathers the needed source rows from its full local copy of H via
    dma_gather (int16 idx => the H table is viewed as 4 row-quarters).
  - Per 128-edge chunk, a one-hot matrix S[e, d] = norm_e * [dstloc_e == d]
    is built on DVE/ACT with a single tensor_scalar (iota is_equal dstloc,
    then mult norm), and PE computes Mt[f, d] += G[e, f]^T... i.e.
    matmul(lhsT=G_chunk, rhs=S_chunk) accumulating into PSUM; this performs
    both the edge scaling and the segment-sum (scatter-add) at once.
  - Mt (= (A @ H)^T for the 128 dst nodes) then goes through the dense
    layer: matmul(lhsT=Mt, rhs=W), bias add + relu, giving the next H tile.
  - Pooling: per tile matmul with a host-built one-hot [node, graph_local]
    accumulates per-graph sums into a persistent PSUM tile; partial sums
    are returned per core and combined on the host (graphs can span cores).
  - Between layers the per-core H shards are AllGathered into a replicated
    [100352, 128] table (layer 3 skips this).

The Bass program is identical across cores (SPMD); all per-core variation
lives in the input arrays.  Per-(tile, quarter) slot capacities are the max
over the 8 cores, rounded up to 128; padded slots gather row 0 with
dstloc = -1 so S kills their contribution.
"""

import math
import os

import numpy as np

import concourse.bacc as bacc
import concourse.bass as bass
import concourse.mybir as mybir
import concourse.tile as tile
from concourse.bass_utils import run_bass_kernel_spmd

F32 = mybir.dt.float32
I16 = mybir.dt.int16

N_NODES = 100000
N_EDGES = 1600000
N_GRAPHS = 256
D = 128
NCORES = 8
P = 128


class Plan:
    """Static layout shared by the program and the per-core data packing."""

    def __init__(self, n_nodes, ncores, tiles_per_block, g_cap, caps):
        self.n_nodes = n_nodes
        self.ncores = ncores
        self.nt = caps.shape[0]            # dst tiles per core
        self.shard = self.nt * P
        self.nq = caps.shape[1]
        self.qrows = -(-(self.shard * ncores) // (self.nq * P)) * P
        self.gb = tiles_per_block
        self.nblocks = math.ceil(self.nt / tiles_per_block)
        self.g_cap = g_cap
        self.caps = caps                   # [nt, 4] slot capacities (mult of 128)

        # slot space: ordered by (block, quarter, tile-within-block)
        self.slot_base = np.zeros((self.nt, self.nq), np.int64)
        pos = 0
        self.block_info = []  # per block: (chb, [(q, qofs_chunks, nidx)], tiles)
        for b in range(self.nblocks):
            tiles = list(range(b * self.gb, min((b + 1) * self.gb, self.nt)))
            qinfo = []
            chb = 0
            for q in range(self.nq):
                nidx = 0
                for t in tiles:
                    self.slot_base[t, q] = pos + nidx
                    nidx += int(self.caps[t, q])
                qinfo.append((q, chb, nidx))
                chb += nidx // P
                pos += nidx
            self.block_info.append((chb, qinfo, tiles))
        self.total_slots = pos
        self.total_chunks = pos // P
        self.max_chb = max(bi[0] for bi in self.block_info)

    def tile_chunks(self, t):
        """Global chunk ids + in-block chunk offsets for dst tile t."""
        b = t // self.gb
        out = []
        for (q, qofs, _nidx) in self.block_info[b][1]:
            s0 = int(self.slot_base[t, q])
            nch = int(self.caps[t, q]) // P
            block_slot0 = sum(
                self.block_info[bb][0] * P for bb in range(b))
            for c in range(nch):
                gchunk = s0 // P + c
                lchunk = (s0 - block_slot0) // P + c
                out.append((gchunk, lchunk))
        return out


def build_plan(col, row, batch, tiles_per_block=2):
    """Compute SPMD-uniform capacities from the actual edge distribution."""
    n = N_NODES
    nt = math.ceil(math.ceil(n / NCORES) / P)
    shard = nt * P
    nq = int(os.environ.get("GCN_QUARTERS", "0")) or (
        1 if shard * NCORES <= 32768 else 4)
    qrows = -(-(shard * NCORES) // (nq * P)) * P
    core = col // shard
    t_local = (col - core * shard) // P
    q = row // qrows
    key = (core * nt + t_local) * nq + q
    counts = np.bincount(key, minlength=NCORES * nt * nq).reshape(
        NCORES, nt, nq)
    caps = counts.max(axis=0)
    caps = ((caps + P - 1) // P) * P
    # every (t, q) group must be non-empty so chunk bookkeeping stays simple
    caps = np.maximum(caps, P)

    # pooling span
    g_cap = 0
    for c in range(NCORES):
        lo = batch[min(c * shard, n - 1)]
        hi = batch[min((c + 1) * shard, n) - 1]
        g_cap = max(g_cap, int(hi - lo + 1))
    g_cap = min(max(((g_cap + 7) // 8) * 8, 16), 128)
    return Plan(n, NCORES, tiles_per_block, g_cap, caps)


def pack_core_data(plan, c, row, col, normv, batch):
    """Pack one core's edge data into the padded slot space."""
    nt, shard, qrows = plan.nt, plan.shard, plan.qrows
    m = (col >= c * shard) & (col < (c + 1) * shard)
    rowc = row[m]
    colc = col[m]
    nrmc = normv[m]
    t_local = (colc - c * shard) // P
    q = rowc // qrows
    dl = colc - (c * shard + t_local * P)

    key = t_local * plan.nq + q
    order = np.argsort(key, kind="stable")
    skey = key[order]
    # rank within group
    grp_start = np.searchsorted(skey, np.arange(nt * plan.nq), side="left")
    rank = np.arange(len(skey)) - grp_start[skey]
    slot = plan.slot_base.reshape(-1)[skey] + rank

    ns = plan.total_slots
    idx_flat = np.zeros(ns, np.int16)
    dst_flat = np.full(ns, -1.0, np.float32)
    nrm_flat = np.zeros(ns, np.float32)
    rel = rowc[order] - q[order] * qrows
    assert rel.min() >= 0 and rel.max() < min(plan.qrows, 32768), (
        rel.min(), rel.max())
    assert dl.min() >= 0 and dl.max() < P
    assert len(np.unique(slot)) == len(slot)
    idx_flat[slot] = rel.astype(np.int16)
    dst_flat[slot] = dl[order].astype(np.float32)
    nrm_flat[slot] = nrmc[order]

    # [128, chunks] layout: slot s -> (s % 128, s // 128)
    dst_sb = np.ascontiguousarray(dst_flat.reshape(-1, P).T)
    nrm_sb = np.ascontiguousarray(nrm_flat.reshape(-1, P).T)
    # idx16 wrap: slot s -> (s % 16, s // 16), replicated to 128 partitions
    idx16 = np.ascontiguousarray(
        np.tile(idx_flat.reshape(-1, 16).T, (8, 1)))

    # pooling one-hot [128, nt * g_cap]
    g_lo = int(batch[min(c * shard, plan.n_nodes - 1)])
    pool = np.zeros((P, nt * plan.g_cap), np.float32)
    n_real = min((c + 1) * shard, plan.n_nodes) - c * shard
    nodes = np.arange(n_real)
    gl = batch[c * shard + nodes] - g_lo
    pool[nodes % P, (nodes // P) * plan.g_cap + gl] = 1.0
    return dict(idx16=idx16, dstloc=dst_sb, normv=nrm_sb, pool=pool), g_lo


def build_program(plan):
    nc = bacc.Bacc(num_devices=NCORES)
    n_pad = plan.shard * NCORES
    tch = plan.total_chunks

    xt_d = nc.dram_tensor("xt", [plan.n_nodes, D], F32, kind="ExternalInput")
    idx_d = nc.dram_tensor("idx16", [P, plan.total_slots // 16], I16,
                           kind="ExternalInput")
    dst_d = nc.dram_tensor("dstloc", [P, tch], F32, kind="ExternalInput")
    nrm_d = nc.dram_tensor("normv", [P, tch], F32, kind="ExternalInput")
    pool_d = nc.dram_tensor("pool", [P, plan.nt * plan.g_cap], F32,
                            kind="ExternalInput")
    # per-layer constants: W [f, f'], bias broadcast [128, 128], iota row
    wb_d = nc.dram_tensor("wb", [P, 3 * D + 3 * D + D], F32,
                          kind="ExternalInput")
    out_d = nc.dram_tensor("pool_out", [3, plan.g_cap, D], F32,
                           kind="ExternalOutput")

    h_own = nc.dram_tensor("h_own", [plan.shard, D], F32)
    ag = [nc.dram_tensor(f"ag{l}", [n_pad, D], F32, addr_space="Shared")
          for l in range(2)]

    def quarters(tensor, nrows):
        qs = []
        for qi in range(plan.nq):
            r0 = qi * plan.qrows
            r1 = min((qi + 1) * plan.qrows, nrows)
            qs.append(tensor[r0:r1, :])
        return qs

    tables = [quarters(xt_d, plan.n_nodes),
              quarters(ag[0], n_pad), quarters(ag[1], n_pad)]

    with tile.TileContext(nc) as tc:
        explicit = bool(os.environ.get("GCN_EXPLICIT_ENGINES"))
        ts_eng = nc.vector if explicit else nc.any
        no_pool = bool(os.environ.get("GCN_NO_POOL"))
        with (
            tc.tile_pool(name="const", bufs=1) as cp,
            tc.tile_pool(name="gpool", bufs=2) as gp,
            tc.tile_pool(name="spool", bufs=2) as spp,
            tc.tile_pool(name="work", bufs=3) as wp,
            tc.tile_pool(name="mt_ps", bufs=4, space="PSUM") as mtp,
            tc.tile_pool(name="h_ps", bufs=2, space="PSUM") as hpp,
            tc.tile_pool(name="pool_ps", bufs=2, space="PSUM") as ppp,
        ):
            idx_sb = cp.tile([P, plan.total_slots // 16], I16)
            nc.sync.dma_start(out=idx_sb[:], in_=idx_d[:])
            dst_sb = cp.tile([P, tch], F32)
            nc.sync.dma_start(out=dst_sb[:], in_=dst_d[:])
            nrm_sb = cp.tile([P, tch], F32)
            nc.sync.dma_start(out=nrm_sb[:], in_=nrm_d[:])
            pool_sb = cp.tile([P, plan.nt * plan.g_cap], F32)
            nc.sync.dma_start(out=pool_sb[:], in_=pool_d[:])
            wb_sb = cp.tile([P, 7 * D], F32)
            nc.sync.dma_start(out=wb_sb[:], in_=wb_d[:])
            w_ap = [wb_sb[:, l * D:(l + 1) * D] for l in range(3)]
            bb_ap = [wb_sb[:, (3 + l) * D:(4 + l) * D] for l in range(3)]
            iota_ap = wb_sb[:, 6 * D:7 * D]

            n_layers = int(os.environ.get("GCN_LAYERS", "3"))
            no_ag = bool(os.environ.get("GCN_NO_AG"))
            for l in range(n_layers):
                pool_ps = ppp.tile([plan.g_cap, D], F32, space="PSUM",
                                   tag="poolps")
                for b in range(plan.nblocks):
                    chb, qinfo, tiles = plan.block_info[b]
                    block_slot0 = sum(
                        plan.block_info[bb][0] * P
                        for bb in range(b))
                    g = gp.tile([P, plan.max_chb * D], F32, tag="g")
                    for (q, qofs, nidx) in qinfo:
                        s0 = block_slot0 + qofs * P
                        nc.gpsimd.dma_gather(
                            out_ap=g[:, qofs * D:(qofs + nidx // P) * D]
                                .rearrange("p (c f) -> p c f", f=D),
                            in_ap=tables[l][q],
                            idxs_ap=idx_sb[:, s0 // 16:(s0 + nidx) // 16],
                            num_idxs=nidx,
                            num_idxs_reg=nidx,
                            elem_size=D,
                            single_packet=False,
                        )
                    s = spp.tile([P, plan.max_chb * D], F32, tag="s")
                    for t in tiles:
                        for (gch, lch) in plan.tile_chunks(t):
                            ts_eng.tensor_scalar(
                                out=s[:, lch * D:(lch + 1) * D],
                                in0=iota_ap,
                                scalar1=dst_sb[:, gch:gch + 1],
                                scalar2=nrm_sb[:, gch:gch + 1],
                                op0=mybir.AluOpType.is_equal,
                                op1=mybir.AluOpType.mult,
                            )
                    for t in tiles:
                        chunks = plan.tile_chunks(t)
                        mt = mtp.tile([P, D], F32, space="PSUM", tag="mt")
                        for i, (gch, lch) in enumerate(chunks):
                            nc.tensor.matmul(
                                out=mt[:],
                                lhsT=g[:, lch * D:(lch + 1) * D],
                                rhs=s[:, lch * D:(lch + 1) * D],
                                start=(i == 0),
                                stop=(i == len(chunks) - 1),
                            )
                        mts = wp.tile([P, D], F32, tag="mts")
                        (nc.scalar.copy(out=mts[:], in_=mt[:]) if explicit
                         else nc.any.tensor_copy(out=mts[:], in_=mt[:]))
                        hp = hpp.tile([P, D], F32, space="PSUM", tag="hps")
                        nc.tensor.matmul(out=hp[:], lhsT=mts[:],
                                         rhs=w_ap[l], start=True, stop=True)
                        hb = wp.tile([P, D], F32, tag="hb")
                        ts_eng.tensor_tensor(out=hb[:], in0=hp[:],
                                             in1=bb_ap[l],
                                             op=mybir.AluOpType.add)
                        ts_eng.tensor_scalar_max(hb[:], hb[:], 0.0)
                        if not no_pool:
                            nc.tensor.matmul(
                                out=pool_ps[:],
                                lhsT=pool_sb[:, t * plan.g_cap:(t + 1) * plan.g_cap],
                                rhs=hb[:],
                                start=(t == 0),
                                stop=(t == plan.nt - 1),
                            )
                        if l < 2:
                            nc.sync.dma_start(
                                out=h_own[t * P:(t + 1) * P, :], in_=hb[:])
                if l < 2 and not no_ag:
                    nc.gpsimd.collective_compute(
                        "AllGather",
                        mybir.AluOpType.bypass,
                        replica_groups=[list(range(NCORES))],
                        ins=[h_own[:]],
                        outs=[ag[l][:]],
                    )
                if not no_pool:
                    pc = wp.tile([plan.g_cap, D], F32, tag="poolout")
                    (nc.scalar.copy(out=pc[:], in_=pool_ps[:]) if explicit
                     else nc.any.tensor_copy(out=pc[:], in_=pool_ps[:]))
                    nc.sync.dma_start(out=out_d[l], in_=pc[:])
    nc.finalize()
    return nc


def kernel(x, edge_index, edge_weight, batch, W1, b1, W2, b2, W3, b3):
    x = np.ascontiguousarray(np.asarray(x, np.float32))
    edge_index = np.asarray(edge_index, np.int64)
    edge_weight = np.asarray(edge_weight, np.float32)
    batch = np.asarray(batch, np.int64)
    n = x.shape[0]

    # normalization coefficients (host: O(E) scalar work)
    row = np.concatenate([edge_index[0], np.arange(n, dtype=np.int64)])
    col = np.concatenate([edge_index[1], np.arange(n, dtype=np.int64)])
    w = np.concatenate([edge_weight, np.ones(n, np.float32)])
    deg = np.bincount(col, weights=w.astype(np.float64), minlength=n)
    dinv = np.where(deg > 0, 1.0 / np.sqrt(deg), 0.0)
    normv = (dinv[row] * w * dinv[col]).astype(np.float32)

    plan = build_plan(col, row, batch)
    nc = build_program(plan)

    iota = np.broadcast_to(np.arange(P, dtype=np.float32), (P, P))
    wb = np.concatenate(
        [np.asarray(W1, np.float32), np.asarray(W2, np.float32),
         np.asarray(W3, np.float32),
         np.broadcast_to(np.asarray(b1, np.float32), (P, D)),
         np.broadcast_to(np.asarray(b2, np.float32), (P, D)),
         np.broadcast_to(np.asarray(b3, np.float32), (P, D)),
         iota], axis=1)
    wb = np.ascontiguousarray(wb)

    in_maps = []
    g_los = []
    for c in range(NCORES):
        data, g_lo = pack_core_data(plan, c, row, col, normv, batch)
        data["xt"] = x
        data["wb"] = wb
        in_maps.append(data)
        g_los.append(g_lo)

    res = run_bass_kernel_spmd(nc, in_maps, list(range(NCORES)),
                               trace=bool(os.environ.get("GCN_TRACE")))
    global LAST_RESULTS
    LAST_RESULTS = res

    counts = np.maximum(np.bincount(batch, minlength=N_GRAPHS), 1.0)
    embs = []
    for l in range(3):
        acc = np.zeros((N_GRAPHS, D), np.float64)
        for c in range(NCORES):
            part = res.results[c]["pool_out"][l]
            lo = g_los[c]
            hi = min(lo + plan.g_cap, N_GRAPHS)
            acc[lo:hi] += part[:hi - lo]
        embs.append((acc / counts[:, None]).astype(np.float32))
    return tuple(embs)



# revision 4
# speedup vs baseline: 2.8739x; 2.8739x over previous
"""GCN block (3x GCNConv(128,128) + relu + global_mean_pool) on 8 trn2 cores.

v2 strategy (same graph partition by destination node as v1, re-engineered
around the measured bottlenecks: Q7 SWDGE descriptor generation, DVE
tensor_scalar S-builds, and their SBUF-port contention):

  - All device-side tensors are bf16 (PSUM accumulation stays fp32).
  - The one-hot scatter matrices S (static across layers!) are precomputed
    on the HOST in bf16 and streamed per block via HWDGE (nc.sync.dma_start)
    instead of being built per chunk on DVE.  DVE does nothing; pointwise
    work runs on the Scalar/ACT engine, so GpSimd's SWDGE descriptor
    generation no longer contends with DVE 2-port SBUF locks.
  - Self-loops are removed from the gather: each core keeps its own H shard
    resident in SBUF (hself) and applies the self-loop contribution as one
    extra chunk matmul against a host-built diagonal S (s_self).  This cuts
    ~12.5K gather descriptors per core per layer and shrinks bucket padding.
  - The bias is folded into the PE as a rank-1 matmul (ones[1,128]^T @
    b[1,128]) accumulating into the same PSUM as the H@W product, so
    bias+relu collapses into a single ACT activation(Relu) with bf16 cast.
  - Gathers go to 4 SWDGE queues (one per source quarter) so descriptor
    rings drain in parallel; blocks of 4 dst tiles per gather amortize the
    per-instruction overhead.

Per-core data layout (SPMD: same program, per-core arrays):
  - nodes split into 8 shards of 12544 (98 tiles of 128)
  - edges bucketed by (dst tile, src quarter), capacities = max over cores
    rounded up to 128; slot order (block, quarter, tile)
  - gather: int16 idx relative to the quarter, [128, slots/16] replicated
  - S stream: [128, total_chunks*128] bf16, chunk c columns = norm one-hot
    for slots [128c, 128c+128)
"""

import math
import os

import ml_dtypes
import numpy as np

import concourse.bacc as bacc
import concourse.bass as bass
import concourse.mybir as mybir
import concourse.tile as tile
from concourse.bass_utils import run_bass_kernel_spmd

F32 = mybir.dt.float32
BF16 = mybir.dt.bfloat16
I16 = mybir.dt.int16

N_NODES = 100000
N_EDGES = 1600000
N_GRAPHS = 256
D = 128
NCORES = 8
P = 128
NQ = 4

BF = ml_dtypes.bfloat16


class Plan:
    def __init__(self, n_nodes, tiles_per_block, g_cap, caps):
        self.n_nodes = n_nodes
        self.nt = caps.shape[0]
        self.shard = self.nt * P
        self.n_pad = self.shard * NCORES
        self.qrows = self.n_pad // NQ
        self.gb = tiles_per_block
        self.nblocks = math.ceil(self.nt / tiles_per_block)
        self.g_cap = g_cap
        self.caps = caps  # [nt, NQ] slot capacities (multiples of 128)

        # slot space ordered by (block, quarter, tile-within-block)
        self.slot_base = np.zeros((self.nt, NQ), np.int64)
        pos = 0
        self.block_info = []  # per block: (chb, [(q, qofs_chunks, nidx)], tiles)
        for b in range(self.nblocks):
            tiles = list(range(b * self.gb, min((b + 1) * self.gb, self.nt)))
            qinfo = []
            chb = 0
            for q in range(NQ):
                nidx = 0
                for t in tiles:
                    self.slot_base[t, q] = pos + nidx
                    nidx += int(self.caps[t, q])
                qinfo.append((q, chb, nidx))
                chb += nidx // P
                pos += nidx
            self.block_info.append((chb, qinfo, tiles))
        self.total_slots = pos
        self.total_chunks = pos // P
        self.max_chb = max(bi[0] for bi in self.block_info)
        self.block_chunk0 = []
        c0 = 0
        for b in range(self.nblocks):
            self.block_chunk0.append(c0)
            c0 += self.block_info[b][0]

    def tile_chunks(self, t):
        """Block-local chunk offsets for dst tile t."""
        b = t // self.gb
        c0 = self.block_chunk0[b]
        out = []
        for (q, qofs, _nidx) in self.block_info[b][1]:
            s0 = int(self.slot_base[t, q])
            nch = int(self.caps[t, q]) // P
            block_slot0 = c0 * P
            for c in range(nch):
                lch = (s0 - block_slot0) // P + c
                out.append(lch)
        return out


def build_plan(col, row, batch, tiles_per_block):
    n = N_NODES
    nt = math.ceil(math.ceil(n / NCORES) / P)
    shard = nt * P
    qrows = shard * NCORES // NQ
    core = col // shard
    t_local = (col - core * shard) // P
    q = row // qrows
    key = (core * nt + t_local) * NQ + q
    counts = np.bincount(key, minlength=NCORES * nt * NQ).reshape(NCORES, nt, NQ)
    caps = counts.max(axis=0)
    caps = np.maximum(((caps + P - 1) // P) * P, P)

    g_cap = 0
    for c in range(NCORES):
        lo = batch[min(c * shard, n - 1)]
        hi = batch[min((c + 1) * shard, n) - 1]
        g_cap = max(g_cap, int(hi - lo + 1))
    g_cap = min(max(((g_cap + 7) // 8) * 8, 16), 128)
    return Plan(n, tiles_per_block, g_cap, caps)


def pack_core_data(plan, c, row, col, normv, norm_self, batch):
    """Pack one core's edge data: gather idx, S stream, s_self, pool."""
    nt, shard, qrows = plan.nt, plan.shard, plan.qrows
    m = (col >= c * shard) & (col < (c + 1) * shard)
    rowc = row[m]
    colc = col[m]
    nrmc = normv[m]
    t_local = (colc - c * shard) // P
    q = rowc // qrows
    dl = colc - (c * shard + t_local * P)

    key = t_local * NQ + q
    order = np.argsort(key, kind="stable")
    skey = key[order]
    grp_start = np.searchsorted(skey, np.arange(nt * NQ), side="left")
    rank = np.arange(len(skey)) - grp_start[skey]
    slot = plan.slot_base.reshape(-1)[skey] + rank

    ns = plan.total_slots
    idx_flat = np.zeros(ns, np.int16)
    rel = rowc[order] - q[order] * qrows
    assert rel.min() >= 0 and rel.max() < min(qrows, 32768)
    idx_flat[slot] = rel.astype(np.int16)
    idx16 = np.ascontiguousarray(np.tile(idx_flat.reshape(-1, 16).T, (8, 1)))

    # S stream [128, total_chunks*128] bf16
    s_flat = np.zeros((ns, P), np.float32)
    s_flat[slot, dl[order]] = nrmc[order]
    nch = plan.total_chunks
    s_sb = np.ascontiguousarray(
        s_flat.reshape(nch, P, P).transpose(1, 0, 2).reshape(P, nch * P)
    ).astype(BF)

    # diagonal self-loop S [128, nt*128] bf16
    s_self = np.zeros((P, nt * P), np.float32)
    n_real = min((c + 1) * shard, plan.n_nodes) - c * shard
    nodes = np.arange(n_real)
    s_self[nodes % P, (nodes // P) * P + nodes % P] = norm_self[c * shard + nodes]
    s_self = np.ascontiguousarray(s_self).astype(BF)

    # pooling one-hot [128, nt * g_cap] bf16
    g_lo = int(batch[min(c * shard, plan.n_nodes - 1)])
    pool = np.zeros((P, nt * plan.g_cap), np.float32)
    gl = batch[c * shard + nodes] - g_lo
    pool[nodes % P, (nodes // P) * plan.g_cap + gl] = 1.0
    pool = np.ascontiguousarray(pool).astype(BF)
    return dict(idx16=idx16, s_stream=s_sb, s_self=s_self, pool=pool), g_lo


def build_program(plan):
    nc = bacc.Bacc(num_devices=NCORES,
                   num_swdge_queues=int(os.environ.get("GCN_QUEUES", "4")))
    tch = plan.total_chunks
    nt = plan.nt
    single_packet = bool(int(os.environ.get("GCN_SP", "0")))

    xt_d = nc.dram_tensor("xt", [plan.n_pad, D], BF16, kind="ExternalInput")
    xself_d = nc.dram_tensor("xself", [P, nt * D], BF16, kind="ExternalInput")
    idx_d = nc.dram_tensor("idx16", [P, plan.total_slots // 16], I16,
                           kind="ExternalInput")
    s_d = nc.dram_tensor("s_stream", [P, tch * D], BF16, kind="ExternalInput")
    sself_d = nc.dram_tensor("s_self", [P, nt * D], BF16, kind="ExternalInput")
    pool_d = nc.dram_tensor("pool", [P, nt * plan.g_cap], BF16,
                            kind="ExternalInput")
    # W1 W2 W3 | bias bcast b1 b2 b3 | ones
    wb_d = nc.dram_tensor("wb", [P, 7 * D], BF16, kind="ExternalInput")
    out_d = nc.dram_tensor("pool_out", [3, plan.g_cap, D], F32,
                           kind="ExternalOutput")

    h_own = nc.dram_tensor("h_own", [plan.shard, D], BF16)
    ag = [nc.dram_tensor(f"ag{l}", [plan.n_pad, D], BF16, addr_space="Shared")
          for l in range(2)]

    def quarters(tensor):
        return [tensor[q * plan.qrows:(q + 1) * plan.qrows, :]
                for q in range(NQ)]

    tables = [quarters(xt_d), quarters(ag[0]), quarters(ag[1])]

    nqueues = nc.num_swdge_queues
    with tile.TileContext(nc) as tc:
        with (
            tc.tile_pool(name="const", bufs=1) as cp,
            tc.tile_pool(name="gpool", bufs=2) as gp,
            tc.tile_pool(name="spool", bufs=2) as spp,
            tc.tile_pool(name="work", bufs=3) as wp,
            tc.tile_pool(name="mt_ps", bufs=4, space="PSUM") as mtp,
            tc.tile_pool(name="h_ps", bufs=2, space="PSUM") as hpp,
            tc.tile_pool(name="pool_ps", bufs=2, space="PSUM") as ppp,
        ):
            idx_sb = cp.tile([P, plan.total_slots // 16], I16)
            nc.sync.dma_start(out=idx_sb[:], in_=idx_d[:])
            sself_sb = cp.tile([P, nt * D], BF16)
            nc.sync.dma_start(out=sself_sb[:], in_=sself_d[:])
            pool_sb = cp.tile([P, nt * plan.g_cap], BF16)
            nc.sync.dma_start(out=pool_sb[:], in_=pool_d[:])
            wb_sb = cp.tile([P, 7 * D], BF16)
            nc.sync.dma_start(out=wb_sb[:], in_=wb_d[:])
            hself = cp.tile([P, nt * D], BF16)
            nc.sync.dma_start(out=hself[:], in_=xself_d[:])
            w_ap = [wb_sb[:, l * D:(l + 1) * D] for l in range(3)]
            brow = [wb_sb[0:1, (3 + l) * D:(4 + l) * D] for l in range(3)]
            ones_row = wb_sb[0:1, 6 * D:7 * D]

            n_layers = int(os.environ.get("GCN_LAYERS", "3"))
            no_ag = bool(os.environ.get("GCN_NO_AG"))
            no_pool = bool(os.environ.get("GCN_NO_POOL"))
            for l in range(n_layers):
                pool_ps = ppp.tile([plan.g_cap, D], F32, space="PSUM",
                                   tag="poolps")
                for b in range(plan.nblocks):
                    chb, qinfo, tiles = plan.block_info[b]
                    block_slot0 = plan.block_chunk0[b] * P
                    g = gp.tile([P, plan.max_chb * D], BF16, tag="g")
                    for (q, qofs, nidx) in qinfo:
                        s0 = block_slot0 + qofs * P
                        nc.gpsimd.dma_gather(
                            out_ap=g[:, qofs * D:(qofs + nidx // P) * D]
                                .rearrange("p (c f) -> p c f", f=D),
                            in_ap=tables[l][q],
                            idxs_ap=idx_sb[:, s0 // 16:(s0 + nidx) // 16],
                            num_idxs=nidx,
                            num_idxs_reg=nidx,
                            elem_size=D,
                            single_packet=single_packet,
                            queue_num=q % nqueues,
                        )
                    s = spp.tile([P, plan.max_chb * D], BF16, tag="s")
                    c0 = plan.block_chunk0[b]
                    nc.sync.dma_start(
                        out=s[:, :chb * D],
                        in_=s_d[:, c0 * D:(c0 + chb) * D])
                    for t in tiles:
                        chunks = plan.tile_chunks(t)
                        mt = mtp.tile([P, D], F32, space="PSUM", tag="mt")
                        nc.tensor.matmul(
                            out=mt[:],
                            lhsT=hself[:, t * D:(t + 1) * D],
                            rhs=sself_sb[:, t * D:(t + 1) * D],
                            start=True, stop=False,
                        )
                        for i, lch in enumerate(chunks):
                            nc.tensor.matmul(
                                out=mt[:],
                                lhsT=g[:, lch * D:(lch + 1) * D],
                                rhs=s[:, lch * D:(lch + 1) * D],
                                start=False,
                                stop=(i == len(chunks) - 1),
                            )
                        mts = wp.tile([P, D], BF16, tag="mts")
                        nc.scalar.copy(out=mts[:], in_=mt[:])
                        hp = hpp.tile([P, D], F32, space="PSUM", tag="hps")
                        nc.tensor.matmul(out=hp[:], lhsT=mts[:], rhs=w_ap[l],
                                         start=True, stop=False)
                        nc.tensor.matmul(out=hp[:], lhsT=ones_row,
                                         rhs=brow[l], start=False, stop=True)
                        if l < 2:
                            hb = hself[:, t * D:(t + 1) * D]
                        else:
                            hb_t = wp.tile([P, D], BF16, tag="hb")
                            hb = hb_t[:]
                        nc.scalar.activation(
                            out=hb, in_=hp[:],
                            func=mybir.ActivationFunctionType.Relu)
                        if not no_pool:
                            nc.tensor.matmul(
                                out=pool_ps[:],
                                lhsT=pool_sb[:, t * plan.g_cap:(t + 1) * plan.g_cap],
                                rhs=hb,
                                start=(t == 0),
                                stop=(t == nt - 1),
                            )
                        if l < 2:
                            nc.sync.dma_start(
                                out=h_own[t * P:(t + 1) * P, :], in_=hb)
                if l < 2 and not no_ag:
                    nc.gpsimd.collective_compute(
                        "AllGather",
                        mybir.AluOpType.bypass,
                        replica_groups=[list(range(NCORES))],
                        ins=[h_own[:]],
                        outs=[ag[l][:]],
                    )
                if not no_pool:
                    pc = wp.tile([plan.g_cap, D], F32, tag="poolout")
                    nc.scalar.copy(out=pc[:], in_=pool_ps[:])
                    nc.sync.dma_start(out=out_d[l], in_=pc[:])
    nc.finalize()
    return nc


def kernel(x, edge_index, edge_weight, batch, W1, b1, W2, b2, W3, b3):
    x = np.asarray(x, np.float32)
    edge_index = np.asarray(edge_index, np.int64)
    edge_weight = np.asarray(edge_weight, np.float32)
    batch = np.asarray(batch, np.int64)
    n = x.shape[0]

    row = edge_index[0]
    col = edge_index[1]
    w = edge_weight
    deg = (np.bincount(col, weights=w.astype(np.float64), minlength=n)
           + 1.0)  # self-loop weight 1
    dinv = 1.0 / np.sqrt(deg)
    normv = (dinv[row] * w * dinv[col]).astype(np.float32)
    norm_self = (dinv * dinv).astype(np.float32)

    gb = int(os.environ.get("GCN_GB", "4"))
    plan = build_plan(col, row, batch, gb)
    nc = build_program(plan)

    x_pad = np.zeros((plan.n_pad, D), np.float32)
    x_pad[:n] = x
    x_bf = x_pad.astype(BF)

    wb = np.concatenate(
        [np.asarray(W1, np.float32), np.asarray(W2, np.float32),
         np.asarray(W3, np.float32),
         np.broadcast_to(np.asarray(b1, np.float32), (P, D)),
         np.broadcast_to(np.asarray(b2, np.float32), (P, D)),
         np.broadcast_to(np.asarray(b3, np.float32), (P, D)),
         np.ones((P, D), np.float32)], axis=1)
    wb = np.ascontiguousarray(wb).astype(BF)

    in_maps = []
    g_los = []
    for c in range(NCORES):
        data, g_lo = pack_core_data(plan, c, row, col, normv, norm_self, batch)
        data["xt"] = x_bf
        xs = x_bf[c * plan.shard:(c + 1) * plan.shard]
        data["xself"] = np.ascontiguousarray(
            xs.reshape(plan.nt, P, D).transpose(1, 0, 2).reshape(P, plan.nt * D))
        data["wb"] = wb
        in_maps.append(data)
        g_los.append(g_lo)

    res = run_bass_kernel_spmd(nc, in_maps, list(range(NCORES)),
                               trace=bool(os.environ.get("GCN_TRACE")))
    global LAST_RESULTS
    LAST_RESULTS = res

    counts = np.maximum(np.bincount(batch, minlength=N_GRAPHS), 1.0)
    embs = []
    for l in range(3):
        acc = np.zeros((N_GRAPHS, D), np.float64)
        for c in range(NCORES):
            part = res.results[c]["pool_out"][l]
            lo = g_los[c]
            hi = min(lo + plan.g_cap, N_GRAPHS)
            acc[lo:hi] += part[:hi - lo]
        embs.append((acc / counts[:, None]).astype(np.float32))
    return tuple(embs)


# revision 15
# speedup vs baseline: 2.9024x; 1.0099x over previous
"""GCN block (3x GCNConv(128,128) + relu + global_mean_pool) on 8 trn2 cores.

v2 strategy (same graph partition by destination node as v1, re-engineered
around the measured bottlenecks: Q7 SWDGE descriptor generation, DVE
tensor_scalar S-builds, and their SBUF-port contention):

  - All device-side tensors are bf16 (PSUM accumulation stays fp32).
  - The one-hot scatter matrices S (static across layers!) are precomputed
    on the HOST in bf16 and streamed per block via HWDGE (nc.sync.dma_start)
    instead of being built per chunk on DVE.  DVE does nothing; pointwise
    work runs on the Scalar/ACT engine, so GpSimd's SWDGE descriptor
    generation no longer contends with DVE 2-port SBUF locks.
  - Self-loops are removed from the gather: each core keeps its own H shard
    resident in SBUF (hself) and applies the self-loop contribution as one
    extra chunk matmul against a host-built diagonal S (s_self).  This cuts
    ~12.5K gather descriptors per core per layer and shrinks bucket padding.
  - The bias is folded into the PE as a rank-1 matmul (ones[1,128]^T @
    b[1,128]) accumulating into the same PSUM as the H@W product, so
    bias+relu collapses into a single ACT activation(Relu) with bf16 cast.
  - Gathers go to 4 SWDGE queues (one per source quarter) so descriptor
    rings drain in parallel; blocks of 4 dst tiles per gather amortize the
    per-instruction overhead.

Per-core data layout (SPMD: same program, per-core arrays):
  - nodes split into 8 shards of 12544 (98 tiles of 128)
  - edges bucketed by (dst tile, src quarter), capacities = max over cores
    rounded up to 128; slot order (block, quarter, tile)
  - gather: int16 idx relative to the quarter, [128, slots/16] replicated
  - S stream: [128, total_chunks*128] bf16, chunk c columns = norm one-hot
    for slots [128c, 128c+128)
"""

import math
import os

import ml_dtypes
import numpy as np

import concourse.bacc as bacc
import concourse.bass as bass
import concourse.mybir as mybir
import concourse.tile as tile
from concourse.bass_utils import run_bass_kernel_spmd

F32 = mybir.dt.float32
BF16 = mybir.dt.bfloat16
I16 = mybir.dt.int16

N_NODES = 100000
N_EDGES = 1600000
N_GRAPHS = 256
D = 128
NCORES = 8
P = 128
NQ = 4

BF = ml_dtypes.bfloat16


class Plan:
    """Table layout (quarter-AllGather friendly):

    node v = c*shard + q*qloc + rr  (c = core, q = quarter, rr < qloc)
    lives at table row  q*qrows + c*qloc + rr  with qloc = shard/4,
    qrows = 2*shard.  Quarter q of the table is then exactly the output
    of AllGather over each core's h_own rows [q*qloc, (q+1)*qloc), so
    layer l+1's quarter-q gathers depend only on quarter-AG q of layer l.
    """

    def __init__(self, n_nodes, tiles_per_block, g_cap, caps):
        self.n_nodes = n_nodes
        self.nt = caps.shape[0]
        self.shard = self.nt * P
        self.n_pad = self.shard * NCORES
        self.qrows = self.n_pad // NQ
        self.qloc = self.shard // NQ
        self.gb = tiles_per_block
        self.nblocks = math.ceil(self.nt / tiles_per_block)
        self.g_cap = g_cap
        self.caps = caps  # [nt, NQ] slot capacities (multiples of 128)

        # slot space ordered by (block, quarter, tile-within-block)
        self.slot_base = np.zeros((self.nt, NQ), np.int64)
        pos = 0
        self.block_info = []  # per block: (chb, [(q, qofs_chunks, nidx)], tiles)
        for b in range(self.nblocks):
            tiles = list(range(b * self.gb, min((b + 1) * self.gb, self.nt)))
            qinfo = []
            chb = 0
            for q in range(NQ):
                nidx = 0
                for t in tiles:
                    self.slot_base[t, q] = pos + nidx
                    nidx += int(self.caps[t, q])
                qinfo.append((q, chb, nidx))
                chb += nidx // P
                pos += nidx
            self.block_info.append((chb, qinfo, tiles))
        self.total_slots = pos
        self.total_chunks = pos // P
        self.max_chb = max(bi[0] for bi in self.block_info)
        self.block_chunk0 = []
        c0 = 0
        for b in range(self.nblocks):
            self.block_chunk0.append(c0)
            c0 += self.block_info[b][0]

    def tile_chunks(self, t):
        """Block-local chunk offsets for dst tile t."""
        b = t // self.gb
        c0 = self.block_chunk0[b]
        out = []
        for (q, qofs, _nidx) in self.block_info[b][1]:
            s0 = int(self.slot_base[t, q])
            nch = int(self.caps[t, q]) // P
            block_slot0 = c0 * P
            for c in range(nch):
                lch = (s0 - block_slot0) // P + c
                out.append(lch)
        return out


def build_plan(col, row, batch, tiles_per_block):
    n = N_NODES
    nt = math.ceil(math.ceil(n / NCORES) / P)
    shard = nt * P
    qloc = shard // NQ
    core = col // shard
    t_local = (col - core * shard) // P
    q = (row // qloc) % NQ
    key = (core * nt + t_local) * NQ + q
    counts = np.bincount(key, minlength=NCORES * nt * NQ).reshape(NCORES, nt, NQ)
    caps = counts.max(axis=0)
    caps = np.maximum(((caps + P - 1) // P) * P, P)

    g_cap = 0
    for c in range(NCORES):
        lo = batch[min(c * shard, n - 1)]
        hi = batch[min((c + 1) * shard, n) - 1]
        g_cap = max(g_cap, int(hi - lo + 1))
    g_cap = min(max(((g_cap + 7) // 8) * 8, 16), 128)
    return Plan(n, tiles_per_block, g_cap, caps)


def pack_core_data(plan, c, row, col, normv, norm_self, batch):
    """Pack one core's edge data: gather idx, S stream, s_self, pool."""
    nt, shard, qloc = plan.nt, plan.shard, plan.qloc
    m = (col >= c * shard) & (col < (c + 1) * shard)
    rowc = row[m]
    colc = col[m]
    nrmc = normv[m]
    t_local = (colc - c * shard) // P
    q = (rowc // qloc) % NQ
    dl = colc - (c * shard + t_local * P)

    key = t_local * NQ + q
    order = np.argsort(key, kind="stable")
    skey = key[order]
    grp_start = np.searchsorted(skey, np.arange(nt * NQ), side="left")
    grp_count = np.bincount(skey, minlength=nt * NQ)
    rank = np.arange(len(skey)) - grp_start[skey]
    slot = plan.slot_base.reshape(-1)[skey] + rank

    ns = plan.total_slots
    idx_flat = np.zeros(ns, np.int16)
    rel = (rowc // shard) * qloc + (rowc % qloc)
    rel = rel[order]
    assert rel.min() >= 0 and rel.max() < min(plan.qrows, 32768)
    idx_flat[slot] = rel.astype(np.int16)
    # Trailing -1 idx would let the SWDGE ucode skip pad descriptors, but the
    # NX-side ring accounting doesn't see the trim and the queue wedges —
    # keep disabled (GCN_TRIM=1 to experiment).
    for b in range(plan.nblocks) if int(os.environ.get("GCN_TRIM", "0")) else []:
        _chb, qinfo, tiles = plan.block_info[b]
        for (qq, _qofs, nidx) in qinfo:
            t_last = tiles[-1]
            fill = int(plan.slot_base[t_last, qq]) + \
                int(grp_count[t_last * NQ + qq])
            seg_end = int(plan.slot_base[tiles[0], qq]) + nidx
            if fill < seg_end:
                idx_flat[fill:seg_end] = -1
    idx16 = np.ascontiguousarray(np.tile(idx_flat.reshape(-1, 16).T, (8, 1)))

    # S stream [128, total_chunks*128] bf16
    s_flat = np.zeros((ns, P), np.float32)
    s_flat[slot, dl[order]] = nrmc[order]
    nch = plan.total_chunks
    s_sb = np.ascontiguousarray(
        s_flat.reshape(nch, P, P).transpose(1, 0, 2).reshape(P, nch * P)
    ).astype(BF)

    # diagonal self-loop S [128, nt*128] bf16
    s_self = np.zeros((P, nt * P), np.float32)
    n_real = min((c + 1) * shard, plan.n_nodes) - c * shard
    nodes = np.arange(n_real)
    s_self[nodes % P, (nodes // P) * P + nodes % P] = norm_self[c * shard + nodes]
    s_self = np.ascontiguousarray(s_self).astype(BF)

    # pooling one-hot [128, nt * g_cap] bf16
    g_lo = int(batch[min(c * shard, plan.n_nodes - 1)])
    pool = np.zeros((P, nt * plan.g_cap), np.float32)
    gl = batch[c * shard + nodes] - g_lo
    pool[nodes % P, (nodes // P) * plan.g_cap + gl] = 1.0
    pool = np.ascontiguousarray(pool).astype(BF)
    return dict(idx16=idx16, s_stream=s_sb, s_self=s_self, pool=pool), g_lo


def build_program(plan):
    nc = bacc.Bacc(num_devices=NCORES,
                   num_swdge_queues=int(os.environ.get("GCN_QUEUES", "4")))
    tch = plan.total_chunks
    nt = plan.nt
    single_packet = bool(int(os.environ.get("GCN_SP", "0")))

    xt_d = nc.dram_tensor("xt", [plan.n_pad, D], BF16, kind="ExternalInput")
    xself_d = nc.dram_tensor("xself", [P, nt * D], BF16, kind="ExternalInput")
    idx_d = nc.dram_tensor("idx16", [P, plan.total_slots // 16], I16,
                           kind="ExternalInput")
    s_d = nc.dram_tensor("s_stream", [P, tch * D], BF16, kind="ExternalInput")
    sself_d = nc.dram_tensor("s_self", [P, nt * D], BF16, kind="ExternalInput")
    pool_d = nc.dram_tensor("pool", [P, nt * plan.g_cap], BF16,
                            kind="ExternalInput")
    # W1 W2 W3 | bias bcast b1 b2 b3 | ones
    wb_d = nc.dram_tensor("wb", [P, 7 * D], BF16, kind="ExternalInput")
    out_d = nc.dram_tensor("pool_out", [3, plan.g_cap, D], F32,
                           kind="ExternalOutput")

    h_own = nc.dram_tensor("h_own", [plan.shard, D], BF16)
    ag = [nc.dram_tensor(f"ag{l}", [plan.n_pad, D], BF16, addr_space="Shared")
          for l in range(2)]

    def quarters(tensor):
        return [tensor[q * plan.qrows:(q + 1) * plan.qrows, :]
                for q in range(NQ)]

    tables = [quarters(xt_d), quarters(ag[0]), quarters(ag[1])]

    nqueues = nc.num_swdge_queues
    with tile.TileContext(nc) as tc:
        with (
            tc.tile_pool(name="const", bufs=1) as cp,
            tc.tile_pool(name="gpool", bufs=2) as gp,
            tc.tile_pool(name="spool", bufs=2) as spp,
            tc.tile_pool(name="work", bufs=3) as wp,
            tc.tile_pool(name="mt_ps", bufs=4, space="PSUM") as mtp,
            tc.tile_pool(name="h_ps", bufs=2, space="PSUM") as hpp,
            tc.tile_pool(name="pool_ps", bufs=2, space="PSUM") as ppp,
        ):
            idx_sb = cp.tile([P, plan.total_slots // 16], I16)
            nc.sync.dma_start(out=idx_sb[:], in_=idx_d[:])
            sself_sb = cp.tile([P, nt * D], BF16)
            nc.sync.dma_start(out=sself_sb[:], in_=sself_d[:])
            pool_sb = cp.tile([P, nt * plan.g_cap], BF16)
            nc.sync.dma_start(out=pool_sb[:], in_=pool_d[:])
            wb_sb = cp.tile([P, 7 * D], BF16)
            nc.sync.dma_start(out=wb_sb[:], in_=wb_d[:])
            hself = cp.tile([P, nt * D], BF16)
            nc.sync.dma_start(out=hself[:], in_=xself_d[:])
            w_ap = [wb_sb[:, l * D:(l + 1) * D] for l in range(3)]
            brow = [wb_sb[0:1, (3 + l) * D:(4 + l) * D] for l in range(3)]
            ones_row = wb_sb[0:1, 6 * D:7 * D]

            # zero both g buffers once: trailing-trimmed gather rows leave
            # whatever was in SBUF, and uninitialized SBUF may hold NaN/Inf
            # bit patterns that would poison the 0-weighted matmul terms.
            for _i in range(2):
                gz = gp.tile([P, plan.max_chb * D], BF16, tag="g")
                nc.vector.memset(gz[:], 0.0)

            # quarter-AG j can fire once all tiles covering h_own rows
            # [j*qloc, (j+1)*qloc) are written
            ag_after = {}
            for j in range(NQ):
                blk = (((j + 1) * plan.qloc - 1) // P) // plan.gb
                ag_after.setdefault(blk, []).append(j)

            n_layers = int(os.environ.get("GCN_LAYERS", "3"))
            no_ag = bool(os.environ.get("GCN_NO_AG"))
            no_pool = bool(os.environ.get("GCN_NO_POOL"))
            qag = bool(int(os.environ.get("GCN_QAG", "1")))
            for l in range(n_layers):
                pool_ps = ppp.tile([plan.g_cap, D], F32, space="PSUM",
                                   tag="poolps")
                for b in range(plan.nblocks):
                    chb, qinfo, tiles = plan.block_info[b]
                    block_slot0 = plan.block_chunk0[b] * P
                    g = gp.tile([P, plan.max_chb * D], BF16, tag="g")
                    for (q, qofs, nidx) in qinfo:
                        s0 = block_slot0 + qofs * P
                        nc.gpsimd.dma_gather(
                            out_ap=g[:, qofs * D:(qofs + nidx // P) * D]
                                .rearrange("p (c f) -> p c f", f=D),
                            in_ap=tables[l][q],
                            idxs_ap=idx_sb[:, s0 // 16:(s0 + nidx) // 16],
                            num_idxs=nidx,
                            num_idxs_reg=nidx,
                            elem_size=D,
                            single_packet=single_packet,
                            queue_num=q % nqueues,
                        )
                    s = spp.tile([P, plan.max_chb * D], BF16, tag="s")
                    c0 = plan.block_chunk0[b]
                    nc.sync.dma_start(
                        out=s[:, :chb * D],
                        in_=s_d[:, c0 * D:(c0 + chb) * D])
                    for t in tiles:
                        chunks = plan.tile_chunks(t)
                        mt = mtp.tile([P, D], F32, space="PSUM", tag="mt")
                        nc.tensor.matmul(
                            out=mt[:],
                            lhsT=hself[:, t * D:(t + 1) * D],
                            rhs=sself_sb[:, t * D:(t + 1) * D],
                            start=True, stop=False,
                        )
                        for i, lch in enumerate(chunks):
                            nc.tensor.matmul(
                                out=mt[:],
                                lhsT=g[:, lch * D:(lch + 1) * D],
                                rhs=s[:, lch * D:(lch + 1) * D],
                                start=False,
                                stop=(i == len(chunks) - 1),
                            )
                        mts = wp.tile([P, D], BF16, tag="mts")
                        nc.scalar.copy(out=mts[:], in_=mt[:])
                        hp = hpp.tile([P, D], F32, space="PSUM", tag="hps")
                        nc.tensor.matmul(out=hp[:], lhsT=mts[:], rhs=w_ap[l],
                                         start=True, stop=False)
                        nc.tensor.matmul(out=hp[:], lhsT=ones_row,
                                         rhs=brow[l], start=False, stop=True)
                        if l < 2:
                            hb = hself[:, t * D:(t + 1) * D]
                        else:
                            hb_t = wp.tile([P, D], BF16, tag="hb")
                            hb = hb_t[:]
                        nc.scalar.activation(
                            out=hb, in_=hp[:],
                            func=mybir.ActivationFunctionType.Relu)
                        if not no_pool:
                            nc.tensor.matmul(
                                out=pool_ps[:],
                                lhsT=pool_sb[:, t * plan.g_cap:(t + 1) * plan.g_cap],
                                rhs=hb,
                                start=(t == 0),
                                stop=(t == nt - 1),
                            )
                        if l < 2:
                            nc.sync.dma_start(
                                out=h_own[t * P:(t + 1) * P, :], in_=hb)
                    if qag and l < 2 and not no_ag and b in ag_after:
                        for j in ag_after[b]:
                            nc.gpsimd.collective_compute(
                                "AllGather",
                                mybir.AluOpType.bypass,
                                replica_groups=[list(range(NCORES))],
                                ins=[h_own[j * plan.qloc:(j + 1) * plan.qloc, :]],
                                outs=[ag[l][j * plan.qrows:(j + 1) * plan.qrows, :]],
                            )
                if not qag and l < 2 and not no_ag:
                    for j in range(NQ):
                        nc.gpsimd.collective_compute(
                            "AllGather",
                            mybir.AluOpType.bypass,
                            replica_groups=[list(range(NCORES))],
                            ins=[h_own[j * plan.qloc:(j + 1) * plan.qloc, :]],
                            outs=[ag[l][j * plan.qrows:(j + 1) * plan.qrows, :]],
                        )
                if not no_pool:
                    pc = wp.tile([plan.g_cap, D], F32, tag="poolout")
                    nc.scalar.copy(out=pc[:], in_=pool_ps[:])
                    nc.sync.dma_start(out=out_d[l], in_=pc[:])
    nc.finalize()
    return nc


def kernel(x, edge_index, edge_weight, batch, W1, b1, W2, b2, W3, b3):
    x = np.asarray(x, np.float32)
    edge_index = np.asarray(edge_index, np.int64)
    edge_weight = np.asarray(edge_weight, np.float32)
    batch = np.asarray(batch, np.int64)
    n = x.shape[0]

    row = edge_index[0]
    col = edge_index[1]
    w = edge_weight
    deg = (np.bincount(col, weights=w.astype(np.float64), minlength=n)
           + 1.0)  # self-loop weight 1
    dinv = 1.0 / np.sqrt(deg)
    normv = (dinv[row] * w * dinv[col]).astype(np.float32)
    norm_self = (dinv * dinv).astype(np.float32)

    gb = int(os.environ.get("GCN_GB", "4"))
    plan = build_plan(col, row, batch, gb)
    nc = build_program(plan)

    x_pad = np.zeros((plan.n_pad, D), np.float32)
    x_pad[:n] = x
    x_bf = x_pad.astype(BF)
    # permute into the quarter-AG table layout (see Plan docstring)
    v = np.arange(plan.n_pad)
    trow = (((v % plan.shard) // plan.qloc) * plan.qrows
            + (v // plan.shard) * plan.qloc + (v % plan.qloc))
    x_tab = np.zeros_like(x_bf)
    x_tab[trow] = x_bf

    wb = np.concatenate(
        [np.asarray(W1, np.float32), np.asarray(W2, np.float32),
         np.asarray(W3, np.float32),
         np.broadcast_to(np.asarray(b1, np.float32), (P, D)),
         np.broadcast_to(np.asarray(b2, np.float32), (P, D)),
         np.broadcast_to(np.asarray(b3, np.float32), (P, D)),
         np.ones((P, D), np.float32)], axis=1)
    wb = np.ascontiguousarray(wb).astype(BF)

    in_maps = []
    g_los = []
    for c in range(NCORES):
        data, g_lo = pack_core_data(plan, c, row, col, normv, norm_self, batch)
        data["xt"] = x_tab
        xs = x_bf[c * plan.shard:(c + 1) * plan.shard]
        data["xself"] = np.ascontiguousarray(
            xs.reshape(plan.nt, P, D).transpose(1, 0, 2).reshape(P, plan.nt * D))
        data["wb"] = wb
        in_maps.append(data)
        g_los.append(g_lo)

    res = run_bass_kernel_spmd(nc, in_maps, list(range(NCORES)),
                               trace=bool(os.environ.get("GCN_TRACE")))
    global LAST_RESULTS
    LAST_RESULTS = res

    counts = np.maximum(np.bincount(batch, minlength=N_GRAPHS), 1.0)
    embs = []
    for l in range(3):
        acc = np.zeros((N_GRAPHS, D), np.float64)
        for c in range(NCORES):
            part = res.results[c]["pool_out"][l]
            lo = g_los[c]
            hi = min(lo + plan.g_cap, N_GRAPHS)
            acc[lo:hi] += part[:hi - lo]
        embs.append((acc / counts[:, None]).astype(np.float32))
    return tuple(embs)
